# revision 45
# baseline (speedup 1.0000x reference)
"""BoundaryLoss Trainium2 kernel (8-core data-parallel).

loss = mean( (softplus(x) - t*x) * w ),  w = 1 + 5*boundary(t > 0.5)
boundary = dilate2(m) & ~erode2(m), 3x3 cross SE, 2 iterations, zero pad.

Reformulation: two iterations of cross erosion/dilation equal one
erosion/dilation by the L1-diamond of radius 2 (13 cells).  With S = the
13-cell sum of the binary mask m (zero padded):
    eroded = [S == 13], dilated = [S >= 1], boundary = [1 <= S <= 12]
    w = 6 - 5*[S == 0 or S == 13]
With a = |5S - 32.5| (ScalarE Abs, affine pre-applied) the whole weight
collapses to ONE extra DVE op:  rm6 = max(a - 33.5, -6) = -w  exactly
(a = 32.5 at S in {0,13} -> rm6 = -1; a <= 27.5 otherwise -> rm6 = -6).
So with s = ln(1+e^x) - t*x and u = rm6*s:
    sum(bce*w) = -sum(u)
summed per-partition via the accum_out rider (negated in the rider so the
accumulator holds +sum(w*s)), masked by a per-strip-kind row-ownership
vector and reduced on the host (the cross-core all-reduce is 8x128 floats).

Per core: 4 images [1024,1024], split into 9 row-strips each.  s0/mid
strips load 128 rows and own 126/124 (vertical halo via the 2-row overlap,
top zero-pad via band-matrix truncation at the partition edge).  The tail
strip loads its last 32 rows bottom-aligned into partitions 96..127
(bottom pad = band truncation at partition 128) and owns rows 994..1023;
partitions 0..95 hold stale-but-finite ring-buffer data whose contribution
the ownership mask kills — so no memsets anywhere.  Tiles pack two
same-kind strips (an image pair) side by side in the free dim (FD=2048);
both halves arrive in ONE SWDGE dma_start (images are DRAM-contiguous) to
halve Q7 descriptor-gen time, keeping the POOL engine a pure DMA feeder.

Engines: S runs on the TensorEngine as 5 PSUM-accumulated band-matrix
matmuls per 512-col section (vertical reach via the band, horizontal reach
via column-shifted rhs windows, clipped at image edges = zero pad).
ScalarE runs Exp, Ln(1+e), Abs — all from ONE activation-table set
(natural_log_exp_and_others; this build's tables have no softplus) — and
is the bottleneck engine at ~86% occupancy.  Inputs are cast fp32->bf16 by
the SWDGE DMA, so every VectorE op runs in a 2x/4x packed mode: mask
threshold, t*x, s, rm6, u as tensor_tensor/tensor_scalar with the row-sum
riding accum_out.  Scheduling notes that mattered: t-DMA before x-DMA
except on job 0 (first Exp starts ~3us earlier), tail jobs last, per-kind
accumulator folds emitted as soon as each kind's block completes, and NO
job splitting / op rebalancing that disturbs the 18-tile rhythm — the PE
drops out of its ramped p-state whenever its matmul stream gaps.

Engine budgets per tile [128,2048] (18 tiles/core, TimelineSim): ACT
5.4us -> 102us (wall), DVE -> 93us, PE -> 82us, POOL -> 39us, DMA ~97us
(HBM floor at 358 GB/s: 2 x 16.8MB fp32 read per core).
"""

import numpy as np
import ml_dtypes

import concourse.bass as bass
import concourse.mybir as mybir
import concourse.tile as tile
from concourse.bass_utils import run_bass_kernel_spmd

F32 = mybir.dt.float32
BF16 = mybir.dt.bfloat16
ALU = mybir.AluOpType
ACT = mybir.ActivationFunctionType

N_CORES = 8
B, H, W = 32, 1024, 1024
B_LOC = B // N_CORES            # 4 images per core


# ---------------------------------------------------------------------------
# Workaround: the neuronxcc walrus build encodes at most one sync-wait per
# instruction; Tile attaches several.  Split them onto single-wait NOPs on
# the same engine right before the instruction (engines execute in order).
def _patched_drain_and_barrier(self, tick_clock, wait_clock):
    from bass_rust import ScopedClock

    nc = self.nc
    probe = nc.sync.nop(hint="tile_tail_wait_probe")
    wait_clock.add_sem_waits(probe.ins, ScopedClock({None: tick_clock.global_clock}))
    waits = list(probe.ins.sync_info.on_wait or [])
    if waits:
        probe.ins.sync_info = mybir.SyncInfo(on_wait=[waits[0]], on_update=[])
        for w in waits[1:]:
            n = nc.sync.nop(hint="tile_tail_wait_split", nofuse=True)
            n.ins.sync_info = mybir.SyncInfo(on_wait=[w], on_update=[])
    nc.sync.drain()
    nc.all_engine_barrier()
    assert self.sems is not None
    popped = nc._tile_sem_poison_stack.pop()
    assert popped is self._sem_poison
    nc.clear_and_free_semaphores(list(self.sems.allocated().values()))
    nc.all_engine_barrier()


tile.TileContext._drain_and_barrier = _patched_drain_and_barrier


def _split_multi_waits(nc: bass.Bass) -> None:
    seen = set()
    nidx = 0
    for ctx in nc.bb_map.values():
        bb = ctx.bb
        if id(bb) in seen:
            continue
        seen.add(id(bb))
        insts = bb.instructions
        i = 0
        while i < len(insts):
            inst = insts[i]
            si = inst.sync_info
            if si is not None and si.on_wait and len(si.on_wait) > 1:
                waits = list(si.on_wait)
                for w in waits[:-1]:
                    nop = mybir.InstNoOp(name=f"I-waitsplit-{nidx}", ins=[], outs=[])
                    nidx += 1
                    nop.engine = inst.engine
                    nop.sync_info = mybir.SyncInfo(on_wait=[w], on_update=[])
                    nc.register_instruction(nop)
                    insts.insert(i, nop)
                    i += 1
                inst.sync_info = mybir.SyncInfo(
                    on_wait=[waits[-1]], on_update=list(si.on_update or [])
                )
            i += 1
# ---------------------------------------------------------------------------


def _band(width: int) -> np.ndarray:
    k = np.arange(128)
    return (np.abs(k[:, None] - k[None, :]) <= width).astype(ml_dtypes.bfloat16)


def _own(lo: int, hi: int) -> np.ndarray:
    v = np.zeros((128, 1), dtype=np.float32)
    v[lo:hi] = 1
    return v


# jobs: (kind, load_row, img_pair) — two same-kind strips per tile.
# "s0": rows 0..127 loaded, owns rows 0..125 (top pad = band truncation)
# "mid": rows a..a+127 loaded, owns a+2..a+125 (a = 124k, k=1..7)
# "tail": rows 992..1023 loaded into partitions 96..127 (bottom pad = band
#   truncation at the partition-128 edge), owns rows 994..1023.  Partitions
#   0..95 hold stale-but-finite data from earlier jobs (tail runs LAST so
#   its ring buffers are initialized); their results are masked by the
#   ownership vector.  This kills the tail memsets entirely.
_JOBS = (
    [("s0", 0, p) for p in ((0, 1), (2, 3))]
    + [("mid", 124 * k, p) for p in ((0, 1), (2, 3)) for k in range(1, 8)]
    + [("tail", 992, p) for p in ((0, 1), (2, 3))]
)
_OWN_RANGES = {"s0": (0, 126), "mid": (2, 126), "tail": (98, 128)}

# Optional half-width (one-image) chunking of selected jobs.  Tried for the
# first/last jobs (faster ramp / shorter tail) and an ACT->DVE abs_max
# rebalance: both perturb the 18-tile rhythm enough that the PE falls out of
# its ramped p-state and the net is a loss, so the set is empty.
_SPLIT_JOBS = set()
_ACC_COL = []
_acc = 0
for _ti in range(len(_JOBS)):
    _ACC_COL.append(_acc)
    _acc += 2 if _ti in _SPLIT_JOBS else 1
_N_ACC = _acc
_KIND_COLS = {
    "s0": (0, _ACC_COL[2]),
    "mid": (_ACC_COL[2], _ACC_COL[16]),
    "tail": (_ACC_COL[16], _N_ACC),
}
# job index after which each kind's accumulator block is complete
_KIND_DONE = {"s0": 1, "mid": 15, "tail": len(_JOBS) - 1}


def build_nc(repeat: int = 1) -> bass.Bass:
    """repeat>1 wraps the tile loop in a HW For_i (timing variant)."""
    import contextlib

    nc = bass.Bass()

    x_d = nc.dram_tensor("inputs", [B_LOC, 1, H, W], F32, kind="ExternalInput")
    t_d = nc.dram_tensor("targets", [B_LOC, 1, H, W], F32, kind="ExternalInput")
    out_d = nc.dram_tensor("out", [128, 1], F32, kind="ExternalOutput")

    band_d = {w: nc.inline_tensor(_band(w), name=f"band{w}") for w in (0, 1, 2)}
    own_d = {k: nc.inline_tensor(_own(*r), name=f"own_{k}") for k, r in _OWN_RANGES.items()}

    n_jobs = len(_JOBS)
    terms = [(0, 2), (-1, 1), (1, 1), (-2, 0), (2, 0)]

    with tile.TileContext(nc) as tc:
        with (
            tc.tile_pool(name="const", bufs=1) as cpool,
            tc.tile_pool(name="acc", bufs=1) as apool,
            tc.tile_pool(name="work", bufs=4) as pool,
            tc.tile_pool(name="psum", bufs=2, space=bass.MemorySpace.PSUM) as psum,
        ):
            bands = {}
            for w in (0, 1, 2):
                bt = cpool.tile([128, 128], BF16, tag=f"band{w}")
                nc.sync.dma_start(bt[:], band_d[w][:])
                bands[w] = bt
            owns = {}
            for k, dten in own_d.items():
                ot = cpool.tile([128, 1], F32, tag=f"own_{k}")
                nc.sync.dma_start(ot[:], dten[:])
                owns[k] = ot
            bias_abs = cpool.tile([128, 1], F32, tag="bias_abs")
            nc.vector.memset(bias_abs[:], -32.5)

            acc_u = apool.tile([128, _N_ACC], F32, tag="acc_u")
            nc.vector.memset(acc_u[:], 0.0)

            out_t = apool.tile([128, 1], F32, tag="out")

            loop_ctx = tc.For_i(0, repeat, 1) if repeat > 1 else contextlib.nullcontext()
            with loop_ctx:
              for ti, (kind, row, pair) in enumerate(_JOBS):
                  t_t = pool.tile([128, 2 * W], BF16, tag="t")
                  x_t = pool.tile([128, 2 * W], BF16, tag="x")
                  m_t = pool.tile([128, 2 * W], BF16, tag="m")
                  a_t = pool.tile([128, 2 * W], BF16, tag="a")
                  e_t = pool.tile([128, 2 * W], BF16, tag="e")
                  sp_t = pool.tile([128, 2 * W], BF16, tag="sp")
                  tx_t = pool.tile([128, 2 * W], BF16, tag="tx")
                  s_t = pool.tile([128, 2 * W], BF16, tag="s")
                  u_t = pool.tile([128, 2 * W], BF16, tag="u")
                  s_ps = psum.tile([128, 2 * W], F32, tag="S")

                  # tail strips land bottom-aligned (partitions 96..127)
                  nrows, p0 = (32, 96) if kind == "tail" else (128, 0)

                  if ti in _SPLIT_JOBS:
                      chunks = [(h, h * W, (h + 1) * W) for h in range(2)]
                  else:
                      chunks = [(None, 0, 2 * W)]

                  for ci, (h, c0c, c1c) in enumerate(chunks):
                      if h is None:
                          i0 = pair[0]
                          t_dst = t_t[p0 : p0 + nrows, :].rearrange("p (i w) -> p i w", i=2)
                          x_dst = x_t[p0 : p0 + nrows, :].rearrange("p (i w) -> p i w", i=2)
                          t_src = t_d[i0 : i0 + 2, 0, row : row + nrows, :].rearrange("i p w -> p i w")
                          x_src = x_d[i0 : i0 + 2, 0, row : row + nrows, :].rearrange("i p w -> p i w")
                          if ti == 0:
                              # x first: the first Exp (ACT = the wall) can
                              # start ~3us earlier; later jobs load t first
                              # (the mask->matmul chain is longer)
                              nc.gpsimd.dma_start(x_dst, x_src)
                              nc.gpsimd.dma_start(t_dst, t_src)
                          else:
                              nc.gpsimd.dma_start(t_dst, t_src)
                              nc.gpsimd.dma_start(x_dst, x_src)
                      else:
                          img = pair[h]
                          nc.gpsimd.dma_start(
                              t_t[p0 : p0 + nrows, c0c:c1c], t_d[img, 0, row : row + nrows, :]
                          )
                          nc.gpsimd.dma_start(
                              x_t[p0 : p0 + nrows, c0c:c1c], x_d[img, 0, row : row + nrows, :]
                          )

                      # binary mask, dense over the chunk
                      nc.vector.tensor_scalar(
                          m_t[:, c0c:c1c], t_t[:, c0c:c1c], 0.5, None, ALU.is_gt
                      )

                      # S = diamond-2 sum: 5 band matmuls per 512-col section,
                      # windows clipped at image columns (= zero padding)
                      for sec in range(c0c // 512, c1c // 512):
                          hbase = (sec // 2) * W
                          o = (sec % 2) * 512
                          for i, (dj, wd) in enumerate(terms):
                              c0 = max(o + dj, 0)
                              c1 = min(o + dj + 512, W)
                              outp = s_ps[:, sec * 512 + c0 - o - dj : sec * 512 + c1 - o - dj]
                              nc.tensor.matmul(
                                  outp,
                                  bands[wd][:],
                                  m_t[:, hbase + c0 : hbase + c1],
                                  start=(i == 0),
                                  stop=(i == len(terms) - 1),
                              )

                      # bce tail: s = ln(1+e^x) - t*x (no usable softplus in
                      # this build's act tables; exp+ln+abs all live in
                      # natural_log_exp_and_others = one table load)
                      nc.scalar.activation(e_t[:, c0c:c1c], x_t[:, c0c:c1c], ACT.Exp)
                      nc.scalar.activation(sp_t[:, c0c:c1c], e_t[:, c0c:c1c], ACT.Ln, bias=1.0)
                      nc.vector.tensor_tensor(
                          tx_t[:, c0c:c1c], t_t[:, c0c:c1c], x_t[:, c0c:c1c], ALU.mult
                      )
                      nc.vector.tensor_tensor(
                          s_t[:, c0c:c1c], sp_t[:, c0c:c1c], tx_t[:, c0c:c1c], ALU.subtract
                      )

                      col = _ACC_COL[ti] + ci
                      # a = |5S - 32.5| (ACT); rm6 = max(a-33.5, -6) = -w
                      # exactly; accum rider negates so the column holds
                      # +sum(w*s)
                      nc.scalar.activation(
                          a_t[:, c0c:c1c], s_ps[:, c0c:c1c], ACT.Abs,
                          bias=bias_abs[:], scale=5.0,
                      )
                      nc.vector.tensor_scalar(
                          a_t[:, c0c:c1c], a_t[:, c0c:c1c], 33.5, -6.0, ALU.subtract, ALU.max
                      )
                      nc.vector.tensor_tensor(
                          u_t[:, c0c:c1c], a_t[:, c0c:c1c], s_t[:, c0c:c1c], ALU.mult
                      )
                      nc.vector.tensor_scalar(
                          u_t[:, c0c:c1c], u_t[:, c0c:c1c], -1.0, None, ALU.mult, ALU.add,
                          accum_out=acc_u[:, col : col + 1],
                      )

                  # fold each kind's accumulator block into out_t as soon as
                  # its last job is emitted: out = sum_kind own_k * block-sum
                  # (keeps only the mid-block fold in the post-loop tail)
                  for kd, done_ti in _KIND_DONE.items():
                      if ti != done_ti:
                          continue
                      lo, hi = _KIND_COLS[kd]
                      ru = apool.tile([128, 1], F32, tag=f"ru_{kd}")
                      nc.vector.tensor_reduce(
                          ru[:], acc_u[:, lo:hi], mybir.AxisListType.X, ALU.add
                      )
                      if ti == _KIND_DONE["s0"]:
                          nc.vector.scalar_tensor_tensor(
                              out_t[:], ru[:], 1.0, owns[kd][:], ALU.mult, ALU.mult
                          )
                      else:
                          d_k = apool.tile([128, 1], F32, tag=f"d_{kd}")
                          nc.vector.scalar_tensor_tensor(
                              d_k[:], ru[:], 1.0, owns[kd][:], ALU.mult, ALU.mult
                          )
                          nc.vector.tensor_tensor(out_t[:], out_t[:], d_k[:], ALU.add)

            nc.sync.dma_start(out_d[:], out_t[:])

    _split_multi_waits(nc)
    nc.finalize()
    return nc


_NC = None


def _get_nc():
    global _NC
    if _NC is None:
        _NC = build_nc()
    return _NC


def kernel(inputs: np.ndarray, targets: np.ndarray) -> np.ndarray:
    nc = _get_nc()
    in_maps = [
        {
            "inputs": np.ascontiguousarray(inputs[c * B_LOC : (c + 1) * B_LOC]),
            "targets": np.ascontiguousarray(targets[c * B_LOC : (c + 1) * B_LOC]),
        }
        for c in range(N_CORES)
    ]
    res = run_bass_kernel_spmd(nc, in_maps, list(range(N_CORES)))
    total = sum(float(r["out"].sum()) for r in res.results)
    return np.float32(total / (B * H * W))
